# revision 18
# baseline (speedup 1.0000x reference)
"""PointConvDensity forward on 8 Trainium2 NeuronCores (Bass/Tile).

Math (see reference): per (b, n, s):
    h[o] = W @ feat + bias;  feat = [pts - c, g - 2c, c, 1/(|g-c|+1e-8)]
    BN(train) over (b,n,s) per channel -> relu -> max over s.

Decomposition:
    h[o,n,s] = base[o,n] + Wu[o]*u[n,s] + Wv[o]*v[n,s]
      base  = Wb @ [points; xyz; ones]   (K=128 GEMM, weight transform on host)
      u     = g - 2c,  v = 1/(|g-c| + 1e-8),  g = xyz[idx]
    With q = sign(gamma) folded into the weights (qh = q*h):
      max_s relu(scale*h + shift) = relu(|scale| * (qb + max_s r2) + shift)
    BN stats from decomposed sums (no pass over the (o,n,s) cube):
      Sh  = q*(S*Sum_n qb + a*Su + b*Sv)
      Sh2 = S*Sum qb^2 + 2(a*qBsu + b*qBsv) + a^2*Suu + b^2*Svv + 2ab*Suv
    where a=q*Wu, b=q*Wv and qBsu[o] = Sum_n qb[o,n]*su[n], su = Sum_s u.
    Cross-core: one 8KB AllReduce of the aggregates, overlapped with the
    main rank-2 loop.

v2 changes vs v1:
  - gather via InstIndirectCopy (resident firmware) instead of ap_gather:
    avoids the ~223us GPSIMD library swap that dominated v1.
  - partition_all_reduce (attn library) replaced by a lib-0 gpsimd C-axis
    reduce + K=1 matmul broadcast: avoids the second ~25us library swap.
  - all matmuls run in fp32r (1 cycle/row at >=256 cols): no bf16 split-K
    machinery, 2-row rank-2 rhs instead of 12 rows.
  - main loop writes 4 matmuls into one 4-bank PSUM tile, one big DVE
    reduce each: fewer instructions, less PSUM-access overhead.
  - program order keeps the AllReduce + finalize off the critical path
    (expand DMAs issue before the collective-dependent readback).
"""

import numpy as np
import ml_dtypes

B, N, S = 8, 2048, 32
OUT = 128
QT = 16              # tiles per expand chunk (2 DMAs per chunk)
BN_EPS = 1e-5
CNT = float(B * N * S)

_CACHE = {}


def _build_nc():
    import concourse.bass as bass
    import concourse.bacc as bacc
    import concourse.tile as tile
    import concourse.mybir as mybir
    from contextlib import ExitStack

    f32 = mybir.dt.float32
    f32r = mybir.dt.float32r
    bf16 = mybir.dt.bfloat16
    AF = mybir.ActivationFunctionType
    ALU = mybir.AluOpType
    AX = mybir.AxisListType

    nc = bacc.Bacc("TRN2", target_bir_lowering=False, debug=False, num_devices=8)

    # ---- DRAM I/O (per-core shapes) ----
    d_gcw = nc.dram_tensor("gcw", [128, 512], f32, kind="ExternalInput").ap()
    d_rbf = nc.dram_tensor("rbf", [128, N], bf16, kind="ExternalInput").ap()
    d_lbf = nc.dram_tensor("lbf", [128, 128], bf16, kind="ExternalInput").ap()
    d_ab = nc.dram_tensor("ab2", [2, 128], bf16, kind="ExternalInput").ap()
    d_cc = nc.dram_tensor("cvec", [128, 16], f32, kind="ExternalInput").ap()
    d_fin = nc.dram_tensor("fin", [128, 8], f32, kind="ExternalInput").ap()
    d_id = nc.dram_tensor("ident", [128, 128], f32, kind="ExternalInput").ap()
    d_one = nc.dram_tensor("ones", [1, 128], f32r, kind="ExternalInput").ap()
    d_out = nc.dram_tensor("out", [N, OUT], f32, kind="ExternalOutput").ap()

    with tile.TileContext(nc) as tc, ExitStack() as ctx:
        sb = ctx.enter_context(tc.tile_pool(name="sb", bufs=1))
        sb2 = ctx.enter_context(tc.tile_pool(name="sb2", bufs=7))
        ps = ctx.enter_context(tc.tile_pool(name="ps", bufs=2, space="PSUM"))
        dram = ctx.enter_context(tc.tile_pool(name="dram", bufs=1, space="DRAM"))

        # ---------- load inputs (u/v deps first) ----------
        t_gc = sb.tile([128, 512], f32, name="gc")
        t_rbf = sb.tile([128, N], bf16, name="rbf")
        t_lbf = sb.tile([128, 128], bf16, name="lbf")
        t_ab = sb.tile([2, 128], bf16, name="ab")
        t_cc = sb.tile([128, 16], f32, name="cc")
        t_fin = sb.tile([128, 8], f32, name="fin")
        t_id = sb.tile([128, 128], f32, name="ident")
        nc.sync.dma_start(t_gc[:, :], d_gcw)
        nc.sync.dma_start(t_rbf[:, :], d_rbf)
        nc.sync.dma_start(t_lbf[:, :], d_lbf)
        nc.sync.dma_start(t_ab[:, :], d_ab)
        nc.sync.dma_start(t_cc[:, :], d_cc)
        nc.sync.dma_start(t_fin[:, :], d_fin)
        nc.sync.dma_start(t_id[:, :], d_id)

        # ---------- u, v on the compact layout ----------
        # stats read the f32 copies; the rank-2 matmul reads bf16 uvS
        uvS = sb.tile([128, 1024], bf16, name="uvS")
        t_uf = sb.tile([128, 512], f32, name="t_uf")
        t_vf = sb.tile([128, 512], f32, name="t_vf")
        t_u = t_uf[:, :]
        t_v = t_vf[:, :]
        cc_b = t_cc[:, :].unsqueeze(2).broadcast_to([128, 16, 32])
        gc3 = t_gc[:, :].rearrange("p (j s) -> p j s", s=32)
        t_t = sb.tile([128, 512], f32, name="t_t")
        t_w = sb.tile([128, 512], f32, name="t_w")
        t3 = t_t[:, :].rearrange("p (j s) -> p j s", s=32)
        nc.vector.tensor_sub(t3, gc3, cc_b)                       # t = g - c
        nc.vector.tensor_sub(t_u.rearrange("p (j s) -> p j s", s=32), t3, cc_b)
        nc.scalar.copy(uvS[:, 0:512], t_u)
        nc.vector.scalar_tensor_tensor(t_w[:, :], t_t[:, :], -1.0, t_t[:, :],
                                       ALU.mult, ALU.max)          # |t|
        nc.vector.tensor_scalar_add(t_w[:, :], t_w[:, :], 1e-8)
        nc.vector.reciprocal(t_v, t_w[:, :])                      # v = 1/(|t|+eps)
        nc.scalar.copy(uvS[:, 512:1024], t_v)

        # ---------- per-core stats ----------
        t_ar = sb.tile([128, 16], f32, name="ar_in")
        nc.vector.memset(t_ar[:, :], 0.0)
        u3v = t_u.rearrange("p (j s) -> p j s", s=32)
        v3v = t_v.rearrange("p (j s) -> p j s", s=32)
        t_su = sb.tile([128, 16], f32r, name="su_seg")
        t_sv = sb.tile([128, 16], f32r, name="sv_seg")
        with nc.allow_low_precision(reason="f32r bytes are f32"):
            nc.vector.tensor_reduce(t_su[:, :], u3v, AX.X, ALU.add)
            nc.vector.tensor_reduce(t_sv[:, :], v3v, AX.X, ALU.add)
        nc.vector.tensor_reduce(t_ar[:, 4:5], t_su[:, :], AX.X, ALU.add)
        nc.vector.tensor_reduce(t_ar[:, 5:6], t_sv[:, :], AX.X, ALU.add)
        # sums of squares / products via ACT accumulator (TTR broken on HW)
        sink_a = sb.tile([128, 512], f32, name="sink_a")
        nc.scalar.activation(sink_a[:, :], t_u, AF.Square, accum_out=t_ar[:, 6:7])
        nc.scalar.activation(sink_a[:, :], t_v, AF.Square, accum_out=t_ar[:, 7:8])
        scr2 = sb.tile([128, N], f32, name="scr2")
        scr = scr2[:, 0:512]
        nc.vector.tensor_mul(scr, t_u, t_v)
        nc.scalar.activation(sink_a[:, :], scr, AF.Copy, accum_out=t_ar[:, 8:9])

        # ---------- base GEMM (fp32r): qb = lbf.T @ rbf ----------
        qb_sb = sb.tile([128, N], f32, name="qb_sb")
        qb_ps = ps.tile([128, 2048], f32, name="bigps")
        for j in range(4):
            sl = slice(j * 512, (j + 1) * 512)
            nc.tensor.matmul(qb_ps[:, sl], t_lbf[:, :], t_rbf[:, sl],
                             start=True, stop=True)
        nc.scalar.copy(qb_sb[:, :], qb_ps[:, :])

        # qb row sums / row sums of squares on the DVE STT accumulator
        sink2 = scr2
        nc.vector.scalar_tensor_tensor(sink2[:, :], qb_sb[:, :], 1.0,
                                       qb_sb[:, :], ALU.mult, ALU.max,
                                       accum_out=t_ar[:, 0:1])
        nc.vector.scalar_tensor_tensor(sink2[:, :], qb_sb[:, :], 1.0,
                                       qb_sb[:, :], ALU.mult, ALU.mult,
                                       accum_out=t_ar[:, 1:2])

        # qBsu / qBsv: su broadcast across partitions via K=1 matmul, acc ACT
        t_rows = sb.tile([1, 2 * N], f32r, name="t_rows")
        t_sur = t_rows[:, 0:N]
        t_svr = t_rows[:, N:2 * N]
        nc.sync.dma_start(t_sur.rearrange("o (p j) -> o p j", j=16), t_su[:, :])
        nc.sync.dma_start(t_svr.rearrange("o (p j) -> o p j", j=16), t_sv[:, :])
        t_one = sb.tile([1, 128], f32r, name="ones")
        nc.sync.dma_start(t_one[:, :], d_one)
        for ci, (t_row, acol) in enumerate(((t_sur, 2), (t_svr, 3))):
            bc_ps = ps.tile([128, 2048], f32, name="bigps")
            for j in range(4):
                sl = slice(j * 512, (j + 1) * 512)
                nc.tensor.matmul(bc_ps[:, sl], t_one[:, :], t_row[:, sl],
                                 start=True, stop=True)
            nc.vector.scalar_tensor_tensor(sink2[:, :], qb_sb[:, :], 1.0,
                                           bc_ps[:, :], ALU.mult, ALU.mult,
                                           accum_out=t_ar[:, acol:acol + 1])

        # ---------- AllReduce of aggregates (overlaps the main loop) ----------
        arA = dram.tile([128, 16], f32, name="arA")
        arB = dram.tile([128, 16], f32, name="arB")
        nc.gpsimd.dma_start(arA[:, :], t_ar[:, :])
        nc.gpsimd.collective_compute(
            "AllReduce", ALU.add,
            replica_groups=[list(range(8))],
            ins=[arA[:, :].opt()],
            outs=[arB[:, :].opt()],
        )

        # ---------- main loop: expand -> K=2 fp32r matmul -> segmented max ----
        t_rmax = sb.tile([128, N], f32, name="rmax")

        uv_bufs = {}

        def expand_dma(q):
            uv_buf = sb2.tile([2, QT * 512], bf16, name="uvq")
            psl = slice(q * QT, (q + 1) * QT)
            for rr in range(2):
                nc.sync.dma_start(uv_buf[rr:rr + 1, :],
                                  uvS[psl, rr * 512:(rr + 1) * 512])
            uv_bufs[q] = uv_buf

        def run_chunk(q):
            uv_buf = uv_bufs[q]
            for gg in range(QT // 4):
                g0 = q * QT + gg * 4          # first tile of this psum group
                r2ps = ps.tile([128, 2048], f32, name="bigps")
                for k in range(4):
                    cb = gg * 4 + k
                    nc.tensor.matmul(r2ps[:, k * 512:(k + 1) * 512],
                                     t_ab[:, :],
                                     uv_buf[:, cb * 512:(cb + 1) * 512],
                                     start=True, stop=True)
                nc.vector.tensor_reduce(
                    t_rmax[:, g0 * 16:(g0 + 4) * 16],
                    r2ps[:, :].rearrange("p (j s) -> p j s", s=32),
                    AX.X, ALU.max)

        for q in range(8):
            expand_dma(q)
        for q in range(8):
            run_chunk(q)

        # ---------- collective readback + cross-partition totals ----------
        t_arg = sb.tile([128, 16], f32, name="ar_out")
        nc.gpsimd.dma_start(t_arg[:, :], arB[:, :])
        # partition totals of Su,Sv,Suu,Svv,Suv via lib-0 gpsimd C-reduce
        t_red1 = sb.tile([1, 8], f32, name="red1")
        nc.gpsimd.tensor_reduce(t_red1[:, 0:5], t_arg[:, 4:9], AX.C, ALU.add)
        # broadcast back to all partitions via K=1 matmul
        t_one32 = sb.tile([1, 128], f32, name="ones32")
        nc.vector.memset(t_one32[:, :], 1.0)
        red_ps = ps.tile([128, 2048], f32, name="bigps")
        nc.tensor.matmul(red_ps[:, 0:8], t_one32[:, :], t_red1[:, :],
                         start=True, stop=True)
        t_red = sb.tile([128, 8], f32, name="ar_red")
        nc.scalar.copy(t_red[:, :], red_ps[:, 0:8])

        # ---------- finalize scale/shift ----------
        def col(t, i):
            return t[:, i:i + 1]

        a_, b_ = col(t_fin, 0), col(t_fin, 1)
        gab, bet = col(t_fin, 2), col(t_fin, 3)
        f1 = sb.tile([128, 12], f32, name="fwork")
        # Sh_pre = S*ar0 + a*Su + b*Sv
        nc.vector.tensor_scalar_mul(col(f1, 0), col(t_arg, 0), float(S))
        nc.vector.tensor_mul(col(f1, 1), a_, col(t_red, 0))
        nc.vector.tensor_mul(col(f1, 2), b_, col(t_red, 1))
        nc.vector.tensor_add(col(f1, 0), col(f1, 0), col(f1, 1))
        nc.vector.tensor_add(col(f1, 0), col(f1, 0), col(f1, 2))   # f1[0] = Sh_pre
        # Sh2 = S*ar1 + 2(a*qBsu + b*qBsv) + a^2*Suu + b^2*Svv + 2ab*Suv
        nc.vector.tensor_scalar_mul(col(f1, 3), col(t_arg, 1), float(S))
        nc.vector.tensor_mul(col(f1, 4), a_, col(t_arg, 2))
        nc.vector.tensor_mul(col(f1, 5), b_, col(t_arg, 3))
        nc.vector.tensor_add(col(f1, 4), col(f1, 4), col(f1, 5))
        nc.vector.tensor_scalar_mul(col(f1, 4), col(f1, 4), 2.0)
        nc.vector.tensor_add(col(f1, 3), col(f1, 3), col(f1, 4))
        nc.vector.tensor_mul(col(f1, 5), a_, a_)
        nc.vector.tensor_mul(col(f1, 5), col(f1, 5), col(t_red, 2))
        nc.vector.tensor_add(col(f1, 3), col(f1, 3), col(f1, 5))
        nc.vector.tensor_mul(col(f1, 5), b_, b_)
        nc.vector.tensor_mul(col(f1, 5), col(f1, 5), col(t_red, 3))
        nc.vector.tensor_add(col(f1, 3), col(f1, 3), col(f1, 5))
        nc.vector.tensor_mul(col(f1, 5), a_, b_)
        nc.vector.tensor_mul(col(f1, 5), col(f1, 5), col(t_red, 4))
        nc.vector.tensor_scalar_mul(col(f1, 5), col(f1, 5), 2.0)
        nc.vector.tensor_add(col(f1, 3), col(f1, 3), col(f1, 5))   # f1[3] = Sh2
        # meanq, var, rs, ascale, shift
        nc.vector.tensor_scalar_mul(col(f1, 6), col(f1, 0), 1.0 / CNT)   # meanq
        nc.vector.tensor_mul(col(f1, 7), col(f1, 6), col(f1, 6))
        nc.vector.tensor_scalar_mul(col(f1, 8), col(f1, 3), 1.0 / CNT)
        nc.vector.tensor_sub(col(f1, 8), col(f1, 8), col(f1, 7))         # var
        t_epsbn = sb.tile([128, 1], f32, name="epsbn")
        nc.vector.memset(t_epsbn[:, :], BN_EPS)
        nc.scalar.activation(col(f1, 9), col(f1, 8), AF.Sqrt, bias=t_epsbn[:, :])
        t_rs = sb.tile([128, 1], f32, name="rs")
        nc.vector.reciprocal(t_rs[:, :], col(f1, 9))
        t_asc = sb.tile([128, 1], f32, name="ascale")
        t_shf = sb.tile([128, 1], f32, name="shift")
        nc.vector.tensor_mul(t_asc[:, :], gab, t_rs[:, :])
        nc.vector.tensor_mul(t_shf[:, :], col(f1, 6), t_asc[:, :])
        nc.vector.tensor_sub(t_shf[:, :], bet, t_shf[:, :])

        # ---------- m = qb + rmax; out = relu(ascale*m + shift); transpose ----
        t_m = scr2
        nc.vector.tensor_add(t_m[:, :], qb_sb[:, :], t_rmax[:, :])
        t_o = sb.tile([128, N], f32, name="ot")
        nc.scalar.activation(t_o[:, :], t_m[:, :], AF.Relu,
                             bias=t_shf[:, :], scale=t_asc[:, :])
        t_ot = sb.tile([128, 16 * 128], f32, name="otT")
        for j in range(16):
            tp_ps = ps.tile([128, 2048], f32, name="bigps")
            nc.tensor.transpose(tp_ps[:, 0:128], t_o[:, j::16], t_id[:, :])
            if j % 2 == 0:
                nc.scalar.copy(t_ot[:, j * 128:(j + 1) * 128], tp_ps[:, 0:128])
            else:
                nc.vector.tensor_scalar_mul(t_ot[:, j * 128:(j + 1) * 128],
                                            tp_ps[:, 0:128], 1.0)
        # out row n = 16p + j lives on partition p: 8KB contiguous per burst
        nc.sync.dma_start(d_out.rearrange("(p j) o -> p (j o)", j=16),
                          t_ot[:, :])

    nc.compile()
    return nc


def _get_nc():
    if "nc" not in _CACHE:
        _CACHE["nc"] = _build_nc()
    return _CACHE["nc"]


def _prep_inputs(xyz, points, idx, W, b, gamma, beta):
    xyz = np.asarray(xyz, np.float32)
    points = np.asarray(points, np.float32)
    idx = np.asarray(idx).astype(np.int64)
    W = np.asarray(W, np.float32)
    b = np.asarray(b, np.float32)
    gamma = np.asarray(gamma, np.float32)
    beta = np.asarray(beta, np.float32)

    D = points.shape[1]
    q = np.where(gamma >= 0, np.float32(1.0), np.float32(-1.0))
    Wpts = W[:, :D]
    Wu = W[:, D]
    Wc = W[:, D + 1] - Wpts.sum(axis=1)
    Wv = W[:, D + 2]
    lhsb = np.zeros((128, 128), np.float32)
    lhsb[:D, :] = q[None, :] * Wpts.T
    lhsb[126, :] = q * Wc
    lhsb[127, :] = q * b

    a_ = (q * Wu).astype(np.float32)
    b_ = (q * Wv).astype(np.float32)
    ab2 = np.stack([a_, b_], axis=0).astype(ml_dtypes.bfloat16)   # [2, 128]

    fin = np.zeros((128, 8), np.float32)
    fin[:, 0] = a_
    fin[:, 1] = b_
    fin[:, 2] = np.abs(gamma)
    fin[:, 3] = beta

    ident = np.eye(128, dtype=np.float32)

    in_maps = []
    for bb in range(B):
        rbf = np.concatenate(
            [points[bb], xyz[bb], np.ones((1, N), np.float32)], axis=0)
        gcw = xyz[bb, 0][idx[bb]].reshape(128, 512).astype(np.float32)
        m = {
            "rbf": rbf.astype(ml_dtypes.bfloat16),
            "lbf": lhsb.astype(ml_dtypes.bfloat16),
            "gcw": np.ascontiguousarray(gcw),
            "ab2": ab2,
            "cvec": np.ascontiguousarray(xyz[bb].reshape(128, 16)),
            "fin": fin,
            "ident": ident,
            "ones": np.ones((1, 128), np.float32),
        }
        in_maps.append(m)
    return in_maps


def kernel(xyz, points, idx, W, b, gamma, beta, _trace=False):
    from concourse.bass_utils import run_bass_kernel_spmd

    nc = _get_nc()
    in_maps = _prep_inputs(xyz, points, idx, W, b, gamma, beta)
    res = run_bass_kernel_spmd(nc, in_maps, core_ids=list(range(8)),
                               trace=_trace)
    if _trace:
        _CACHE["last_results"] = res
    out = np.stack([res.results[c]["out"] for c in range(8)], axis=0)
    return out


# revision 19
# speedup vs baseline: 1.0349x; 1.0349x over previous
"""PointConvDensity forward on 8 Trainium2 NeuronCores (Bass/Tile).

Math (see reference): per (b, n, s):
    h[o] = W @ feat + bias;  feat = [pts - c, g - 2c, c, 1/(|g-c|+1e-8)]
    BN(train) over (b,n,s) per channel -> relu -> max over s.

Decomposition:
    h[o,n,s] = base[o,n] + Wu[o]*u[n,s] + Wv[o]*v[n,s]
      base  = Wb @ [points; xyz; ones]   (K=128 GEMM, weight transform on host)
      u     = g - 2c,  v = 1/(|g-c| + 1e-8),  g = xyz[idx]
    With q = sign(gamma) folded into the weights (qh = q*h):
      max_s relu(scale*h + shift) = relu(|scale| * (qb + max_s r2) + shift)
    BN stats from decomposed sums (no pass over the (o,n,s) cube):
      Sh  = q*(S*Sum_n qb + a*Su + b*Sv)
      Sh2 = S*Sum qb^2 + 2(a*qBsu + b*qBsv) + a^2*Suu + b^2*Svv + 2ab*Suv
    where a=q*Wu, b=q*Wv and qBsu[o] = Sum_n qb[o,n]*su[n], su = Sum_s u.
    Cross-core: one 8KB AllReduce of the aggregates, overlapped with the
    main rank-2 loop.

v2 changes vs v1:
  - gather via InstIndirectCopy (resident firmware) instead of ap_gather:
    avoids the ~223us GPSIMD library swap that dominated v1.
  - partition_all_reduce (attn library) replaced by a lib-0 gpsimd C-axis
    reduce + K=1 matmul broadcast: avoids the second ~25us library swap.
  - all matmuls run in fp32r (1 cycle/row at >=256 cols): no bf16 split-K
    machinery, 2-row rank-2 rhs instead of 12 rows.
  - main loop writes 4 matmuls into one 4-bank PSUM tile, one big DVE
    reduce each: fewer instructions, less PSUM-access overhead.
  - program order keeps the AllReduce + finalize off the critical path
    (expand DMAs issue before the collective-dependent readback).
"""

import numpy as np
import ml_dtypes

B, N, S = 8, 2048, 32
OUT = 128
QT = 16              # tiles per expand chunk (2 DMAs per chunk)
BN_EPS = 1e-5
CNT = float(B * N * S)

_CACHE = {}


def _build_nc():
    import concourse.bass as bass
    import concourse.bacc as bacc
    import concourse.tile as tile
    import concourse.mybir as mybir
    from contextlib import ExitStack

    f32 = mybir.dt.float32
    f32r = mybir.dt.float32r
    bf16 = mybir.dt.bfloat16
    AF = mybir.ActivationFunctionType
    ALU = mybir.AluOpType
    AX = mybir.AxisListType

    nc = bacc.Bacc("TRN2", target_bir_lowering=False, debug=False, num_devices=8)

    # ---- DRAM I/O (per-core shapes) ----
    d_gcw = nc.dram_tensor("gcw", [128, 512], f32, kind="ExternalInput").ap()
    d_rbf = nc.dram_tensor("rbf", [128, N], bf16, kind="ExternalInput").ap()
    d_lbf = nc.dram_tensor("lbf", [128, 128], bf16, kind="ExternalInput").ap()
    d_ab = nc.dram_tensor("ab2", [2, 128], bf16, kind="ExternalInput").ap()
    d_cc = nc.dram_tensor("cvec", [128, 16], f32, kind="ExternalInput").ap()
    d_fin = nc.dram_tensor("fin", [128, 8], f32, kind="ExternalInput").ap()
    d_id = nc.dram_tensor("ident", [128, 128], f32, kind="ExternalInput").ap()
    d_one = nc.dram_tensor("ones", [1, 128], f32r, kind="ExternalInput").ap()
    d_out = nc.dram_tensor("out", [N, OUT], f32, kind="ExternalOutput").ap()

    with tile.TileContext(nc) as tc, ExitStack() as ctx:
        sb = ctx.enter_context(tc.tile_pool(name="sb", bufs=1))
        sb2 = ctx.enter_context(tc.tile_pool(name="sb2", bufs=7))
        ps = ctx.enter_context(tc.tile_pool(name="ps", bufs=2, space="PSUM"))
        dram = ctx.enter_context(tc.tile_pool(name="dram", bufs=1, space="DRAM"))

        # ---------- load inputs (u/v deps first) ----------
        t_gc = sb.tile([128, 512], f32, name="gc")
        t_rbf = sb.tile([128, N], bf16, name="rbf")
        t_lbf = sb.tile([128, 128], bf16, name="lbf")
        t_ab = sb.tile([2, 128], bf16, name="ab")
        t_cc = sb.tile([128, 16], f32, name="cc")
        t_fin = sb.tile([128, 8], f32, name="fin")
        t_id = sb.tile([128, 128], f32, name="ident")
        nc.sync.dma_start(t_gc[:, :], d_gcw)
        nc.sync.dma_start(t_rbf[:, :], d_rbf)
        nc.sync.dma_start(t_lbf[:, :], d_lbf)
        nc.sync.dma_start(t_ab[:, :], d_ab)
        nc.sync.dma_start(t_cc[:, :], d_cc)
        nc.sync.dma_start(t_fin[:, :], d_fin)
        nc.sync.dma_start(t_id[:, :], d_id)

        # ---------- u, v on the compact layout ----------
        # stats read the f32 copies; the rank-2 matmul reads bf16 uvS
        uvS = sb.tile([128, 1024], bf16, name="uvS")
        t_uf = sb.tile([128, 512], f32, name="t_uf")
        t_vf = sb.tile([128, 512], f32, name="t_vf")
        t_u = t_uf[:, :]
        t_v = t_vf[:, :]
        cc_b = t_cc[:, :].unsqueeze(2).broadcast_to([128, 16, 32])
        gc3 = t_gc[:, :].rearrange("p (j s) -> p j s", s=32)
        t_t = sb.tile([128, 512], f32, name="t_t")
        t_w = sb.tile([128, 512], f32, name="t_w")
        t3 = t_t[:, :].rearrange("p (j s) -> p j s", s=32)
        nc.vector.tensor_sub(t3, gc3, cc_b)                       # t = g - c
        nc.vector.tensor_sub(t_u.rearrange("p (j s) -> p j s", s=32), t3, cc_b)
        nc.scalar.copy(uvS[:, 0:512], t_u)
        nc.vector.scalar_tensor_tensor(t_w[:, :], t_t[:, :], -1.0, t_t[:, :],
                                       ALU.mult, ALU.max)          # |t|
        nc.vector.tensor_scalar_add(t_w[:, :], t_w[:, :], 1e-8)
        nc.vector.reciprocal(t_v, t_w[:, :])                      # v = 1/(|t|+eps)
        nc.scalar.copy(uvS[:, 512:1024], t_v)

        # ---------- per-core stats ----------
        t_ar = sb.tile([128, 16], f32, name="ar_in")
        nc.vector.memset(t_ar[:, :], 0.0)
        u3v = t_u.rearrange("p (j s) -> p j s", s=32)
        v3v = t_v.rearrange("p (j s) -> p j s", s=32)
        t_su = sb.tile([128, 16], f32r, name="su_seg")
        t_sv = sb.tile([128, 16], f32r, name="sv_seg")
        with nc.allow_low_precision(reason="f32r bytes are f32"):
            nc.vector.tensor_reduce(t_su[:, :], u3v, AX.X, ALU.add)
            nc.vector.tensor_reduce(t_sv[:, :], v3v, AX.X, ALU.add)
        nc.vector.tensor_reduce(t_ar[:, 4:5], t_su[:, :], AX.X, ALU.add)
        nc.vector.tensor_reduce(t_ar[:, 5:6], t_sv[:, :], AX.X, ALU.add)
        # sums of squares / products via ACT accumulator (TTR broken on HW)
        sink_a = sb.tile([128, 512], f32, name="sink_a")
        nc.scalar.activation(sink_a[:, :], t_u, AF.Square, accum_out=t_ar[:, 6:7])
        nc.scalar.activation(sink_a[:, :], t_v, AF.Square, accum_out=t_ar[:, 7:8])
        scr2 = sb.tile([128, N], f32, name="scr2")
        scr = scr2[:, 0:512]
        nc.vector.tensor_mul(scr, t_u, t_v)
        nc.scalar.activation(sink_a[:, :], scr, AF.Copy, accum_out=t_ar[:, 8:9])

        # ---------- base GEMM (fp32r): qb = lbf.T @ rbf ----------
        qb_sb = sb.tile([128, N], f32, name="qb_sb")
        qb_ps = ps.tile([128, 2048], f32, name="bigps")
        for j in range(4):
            sl = slice(j * 512, (j + 1) * 512)
            nc.tensor.matmul(qb_ps[:, sl], t_lbf[:, :], t_rbf[:, sl],
                             start=True, stop=True)
        nc.scalar.copy(qb_sb[:, :], qb_ps[:, :])

        # qb row sums / row sums of squares on the DVE STT accumulator
        sink2 = scr2
        nc.vector.scalar_tensor_tensor(sink2[:, :], qb_sb[:, :], 1.0,
                                       qb_sb[:, :], ALU.mult, ALU.max,
                                       accum_out=t_ar[:, 0:1])
        nc.vector.scalar_tensor_tensor(sink2[:, :], qb_sb[:, :], 1.0,
                                       qb_sb[:, :], ALU.mult, ALU.mult,
                                       accum_out=t_ar[:, 1:2])

        # qBsu / qBsv: su broadcast across partitions via K=1 matmul, acc ACT
        t_rows = sb.tile([1, 2 * N], f32r, name="t_rows")
        t_sur = t_rows[:, 0:N]
        t_svr = t_rows[:, N:2 * N]
        nc.sync.dma_start(t_sur.rearrange("o (p j) -> o p j", j=16), t_su[:, :])
        nc.sync.dma_start(t_svr.rearrange("o (p j) -> o p j", j=16), t_sv[:, :])
        t_one = sb.tile([1, 128], f32r, name="ones")
        nc.sync.dma_start(t_one[:, :], d_one)
        for ci, (t_row, acol) in enumerate(((t_sur, 2), (t_svr, 3))):
            bc_ps = ps.tile([128, 2048], f32, name="bigps")
            for j in range(4):
                sl = slice(j * 512, (j + 1) * 512)
                nc.tensor.matmul(bc_ps[:, sl], t_one[:, :], t_row[:, sl],
                                 start=True, stop=True)
            nc.vector.scalar_tensor_tensor(sink2[:, :], qb_sb[:, :], 1.0,
                                           bc_ps[:, :], ALU.mult, ALU.mult,
                                           accum_out=t_ar[:, acol:acol + 1])

        # ---------- AllReduce of aggregates (overlaps the main loop) ----------
        arA = dram.tile([128, 16], f32, name="arA")
        arB = dram.tile([128, 16], f32, name="arB")
        nc.gpsimd.dma_start(arA[:, :], t_ar[:, :])
        nc.gpsimd.collective_compute(
            "AllReduce", ALU.add,
            replica_groups=[list(range(8))],
            ins=[arA[:, :].opt()],
            outs=[arB[:, :].opt()],
        )

        # ---------- main loop: expand -> K=2 fp32r matmul -> segmented max ----
        t_rmax = sb.tile([128, N], f32, name="rmax")

        uv_bufs = {}

        def expand_dma(q):
            uv_buf = sb2.tile([2, QT * 512], bf16, name="uvq")
            psl = slice(q * QT, (q + 1) * QT)
            for rr in range(2):
                nc.sync.dma_start(uv_buf[rr:rr + 1, :],
                                  uvS[psl, rr * 512:(rr + 1) * 512])
            uv_bufs[q] = uv_buf

        def run_chunk(q):
            uv_buf = uv_bufs[q]
            for gg in range(QT // 4):
                g0 = q * QT + gg * 4          # first tile of this psum group
                r2ps = ps.tile([128, 2048], f32, name="bigps")
                for k in range(4):
                    cb = gg * 4 + k
                    nc.tensor.matmul(r2ps[:, k * 512:(k + 1) * 512],
                                     t_ab[:, :],
                                     uv_buf[:, cb * 512:(cb + 1) * 512],
                                     start=True, stop=True)
                nc.vector.tensor_reduce(
                    t_rmax[:, g0 * 16:(g0 + 4) * 16],
                    r2ps[:, :].rearrange("p (j s) -> p j s", s=32),
                    AX.X, ALU.max)

        for q in range(8):
            expand_dma(q)
        for q in range(4):
            run_chunk(q)

        # ---------- collective readback + cross-partition totals ----------
        t_arg = sb.tile([128, 16], f32, name="ar_out")
        nc.gpsimd.dma_start(t_arg[:, :], arB[:, :])
        # partition totals of Su,Sv,Suu,Svv,Suv via lib-0 gpsimd C-reduce
        t_red1 = sb.tile([1, 8], f32, name="red1")
        nc.gpsimd.tensor_reduce(t_red1[:, 0:5], t_arg[:, 4:9], AX.C, ALU.add)
        # broadcast back to all partitions via K=1 matmul
        t_one32 = sb.tile([1, 128], f32, name="ones32")
        nc.vector.memset(t_one32[:, :], 1.0)
        red_ps = ps.tile([128, 2048], f32, name="bigps")
        nc.tensor.matmul(red_ps[:, 0:8], t_one32[:, :], t_red1[:, :],
                         start=True, stop=True)
        t_red = sb.tile([128, 8], f32, name="ar_red")
        nc.scalar.copy(t_red[:, :], red_ps[:, 0:8])

        # ---------- finalize scale/shift ----------
        def col(t, i):
            return t[:, i:i + 1]

        a_, b_ = col(t_fin, 0), col(t_fin, 1)
        gab, bet = col(t_fin, 2), col(t_fin, 3)
        f1 = sb.tile([128, 12], f32, name="fwork")
        # Sh_pre = S*ar0 + a*Su + b*Sv
        nc.vector.tensor_scalar_mul(col(f1, 0), col(t_arg, 0), float(S))
        nc.vector.tensor_mul(col(f1, 1), a_, col(t_red, 0))
        nc.vector.tensor_mul(col(f1, 2), b_, col(t_red, 1))
        nc.vector.tensor_add(col(f1, 0), col(f1, 0), col(f1, 1))
        nc.vector.tensor_add(col(f1, 0), col(f1, 0), col(f1, 2))   # f1[0] = Sh_pre
        # Sh2 = S*ar1 + 2(a*qBsu + b*qBsv) + a^2*Suu + b^2*Svv + 2ab*Suv
        nc.vector.tensor_scalar_mul(col(f1, 3), col(t_arg, 1), float(S))
        nc.vector.tensor_mul(col(f1, 4), a_, col(t_arg, 2))
        nc.vector.tensor_mul(col(f1, 5), b_, col(t_arg, 3))
        nc.vector.tensor_add(col(f1, 4), col(f1, 4), col(f1, 5))
        nc.vector.tensor_scalar_mul(col(f1, 4), col(f1, 4), 2.0)
        nc.vector.tensor_add(col(f1, 3), col(f1, 3), col(f1, 4))
        nc.vector.tensor_mul(col(f1, 5), a_, a_)
        nc.vector.tensor_mul(col(f1, 5), col(f1, 5), col(t_red, 2))
        nc.vector.tensor_add(col(f1, 3), col(f1, 3), col(f1, 5))
        nc.vector.tensor_mul(col(f1, 5), b_, b_)
        nc.vector.tensor_mul(col(f1, 5), col(f1, 5), col(t_red, 3))
        nc.vector.tensor_add(col(f1, 3), col(f1, 3), col(f1, 5))
        nc.vector.tensor_mul(col(f1, 5), a_, b_)
        nc.vector.tensor_mul(col(f1, 5), col(f1, 5), col(t_red, 4))
        nc.vector.tensor_scalar_mul(col(f1, 5), col(f1, 5), 2.0)
        nc.vector.tensor_add(col(f1, 3), col(f1, 3), col(f1, 5))   # f1[3] = Sh2
        # meanq, var, rs, ascale, shift
        nc.vector.tensor_scalar_mul(col(f1, 6), col(f1, 0), 1.0 / CNT)   # meanq
        nc.vector.tensor_mul(col(f1, 7), col(f1, 6), col(f1, 6))
        nc.vector.tensor_scalar_mul(col(f1, 8), col(f1, 3), 1.0 / CNT)
        nc.vector.tensor_sub(col(f1, 8), col(f1, 8), col(f1, 7))         # var
        t_epsbn = sb.tile([128, 1], f32, name="epsbn")
        nc.vector.memset(t_epsbn[:, :], BN_EPS)
        nc.scalar.activation(col(f1, 9), col(f1, 8), AF.Sqrt, bias=t_epsbn[:, :])
        t_rs = sb.tile([128, 1], f32, name="rs")
        nc.vector.reciprocal(t_rs[:, :], col(f1, 9))
        t_asc = sb.tile([128, 1], f32, name="ascale")
        t_shf = sb.tile([128, 1], f32, name="shift")
        nc.vector.tensor_mul(t_asc[:, :], gab, t_rs[:, :])
        nc.vector.tensor_mul(t_shf[:, :], col(f1, 6), t_asc[:, :])
        nc.vector.tensor_sub(t_shf[:, :], bet, t_shf[:, :])

        for q in range(4, 8):
            run_chunk(q)

        # ---------- m = qb + rmax; out = relu(ascale*m + shift); transpose ----
        t_m = scr2
        nc.vector.tensor_add(t_m[:, :], qb_sb[:, :], t_rmax[:, :])
        t_o = sb.tile([128, N], f32, name="ot")
        nc.scalar.activation(t_o[:, :], t_m[:, :], AF.Relu,
                             bias=t_shf[:, :], scale=t_asc[:, :])
        t_ot = sb.tile([128, 16 * 128], f32, name="otT")
        for j in range(16):
            tp_ps = ps.tile([128, 2048], f32, name="bigps")
            nc.tensor.transpose(tp_ps[:, 0:128], t_o[:, j::16], t_id[:, :])
            if j % 2 == 0:
                nc.scalar.copy(t_ot[:, j * 128:(j + 1) * 128], tp_ps[:, 0:128])
            else:
                nc.vector.tensor_scalar_mul(t_ot[:, j * 128:(j + 1) * 128],
                                            tp_ps[:, 0:128], 1.0)
        # out row n = 16p + j lives on partition p: 8KB contiguous per burst
        nc.sync.dma_start(d_out.rearrange("(p j) o -> p (j o)", j=16),
                          t_ot[:, :])

    nc.compile()
    return nc


def _get_nc():
    if "nc" not in _CACHE:
        _CACHE["nc"] = _build_nc()
    return _CACHE["nc"]


def _prep_inputs(xyz, points, idx, W, b, gamma, beta):
    xyz = np.asarray(xyz, np.float32)
    points = np.asarray(points, np.float32)
    idx = np.asarray(idx).astype(np.int64)
    W = np.asarray(W, np.float32)
    b = np.asarray(b, np.float32)
    gamma = np.asarray(gamma, np.float32)
    beta = np.asarray(beta, np.float32)

    D = points.shape[1]
    q = np.where(gamma >= 0, np.float32(1.0), np.float32(-1.0))
    Wpts = W[:, :D]
    Wu = W[:, D]
    Wc = W[:, D + 1] - Wpts.sum(axis=1)
    Wv = W[:, D + 2]
    lhsb = np.zeros((128, 128), np.float32)
    lhsb[:D, :] = q[None, :] * Wpts.T
    lhsb[126, :] = q * Wc
    lhsb[127, :] = q * b

    a_ = (q * Wu).astype(np.float32)
    b_ = (q * Wv).astype(np.float32)
    ab2 = np.stack([a_, b_], axis=0).astype(ml_dtypes.bfloat16)   # [2, 128]

    fin = np.zeros((128, 8), np.float32)
    fin[:, 0] = a_
    fin[:, 1] = b_
    fin[:, 2] = np.abs(gamma)
    fin[:, 3] = beta

    ident = np.eye(128, dtype=np.float32)

    in_maps = []
    for bb in range(B):
        rbf = np.concatenate(
            [points[bb], xyz[bb], np.ones((1, N), np.float32)], axis=0)
        gcw = xyz[bb, 0][idx[bb]].reshape(128, 512).astype(np.float32)
        m = {
            "rbf": rbf.astype(ml_dtypes.bfloat16),
            "lbf": lhsb.astype(ml_dtypes.bfloat16),
            "gcw": np.ascontiguousarray(gcw),
            "ab2": ab2,
            "cvec": np.ascontiguousarray(xyz[bb].reshape(128, 16)),
            "fin": fin,
            "ident": ident,
            "ones": np.ones((1, 128), np.float32),
        }
        in_maps.append(m)
    return in_maps


def kernel(xyz, points, idx, W, b, gamma, beta, _trace=False):
    from concourse.bass_utils import run_bass_kernel_spmd

    nc = _get_nc()
    in_maps = _prep_inputs(xyz, points, idx, W, b, gamma, beta)
    res = run_bass_kernel_spmd(nc, in_maps, core_ids=list(range(8)),
                               trace=_trace)
    if _trace:
        _CACHE["last_results"] = res
    out = np.stack([res.results[c]["out"] for c in range(8)], axis=0)
    return out
